# revision 4
# baseline (speedup 1.0000x reference)
"""Trainium2 Bass kernel for nn_AudioSNN: 2-layer spiking NN (snntorch Leaky).

Reference semantics per timestep t (over T=200 steps):
    cur1 = x_t @ w1.T + b1                      # [B, 128]
    m1   = 0.9*m1 + cur1 - (m1_prev > 1)        # reset-by-subtract
    spk1 = (m1 > 1)
    cur2 = spk1 @ w2.T + b2                     # [B, 5]
    m2   = 0.9*m2 + cur2 - (m2_prev > 1)
    out[t] = spk2 = (m2 > 1)

Strategy (pure data-parallel over batch, 8 cores x 1024 batch rows):
  - Transposed layout: states kept as [feature, batch] so H=128 sits on
    SBUF partitions and batch on the free dim.
  - MERGED membrane update: ONE custom DVE op per step processes the
    concatenated [m1(1024) | m2(256)] span.  The DVE is the bottleneck
    engine (fp32 PSUM-source ops run at 1x = ~1 elem/cycle/lane), so
    fusing the two updates saves one op's fixed overhead (~130ns/step).
    Layout requirements this imposes:
      * p1 (mm1 out, [128,1024]) and p2 (mm2 out, [128,256]) live in ONE
        contiguous PSUM span [128,1280] (3 banks; 2 ping-pong bufs).
      * m-state lives in a 16-slot SBUF arena; slot s = [m1(s) | m2(s-2)].
        The 2-step layer-2 lag makes the fusion non-circular:
        in0 = slot(s-1) = [m1(s-1) | m2(s-3)] is exactly what the update
        needs, and the chain merged->sign1->mm2->merged(+2) spans two
        pipeline periods.
      * The m2 bias correction (corr = 0.5*sum(w2)+b2, from the +/-1 spike
        encoding) becomes the DVE op's per-partition s0 bias, which also
        (wrongly) lands on the m1 columns; two extra all-ones K-rows in
        the mm1 stack deposit exactly -(corr_hi+corr_lo) there to cancel
        it (s0 is set to float32(corr_hi)+float32(corr_lo) so the
        cancellation is exact at f32 grade).
      * Pre-step m2 slots (m2(-2), m2(-1)) must come out as zero despite
        the s0 bias: the two PSUM m2-regions are pre-loaded with -s0 via
        a one-time DMA.
  - All matmuls run in fp16 with hi/lo split pairs (x = xh + xl exactly
    to ~2^-22 rel; w likewise), accumulated exactly in fp32 PSUM:
    mm1 = wh@xh + wh@xl + wl@xh (one K=122-stacked pass incl. ones rows,
    two N=512 halves); mm2 = w2h@sg + w2l@sg (2 passes, col-tiled 4x,
    the 4 groups run concurrently in disjoint 32-col PE tiles).
  - Spikes are encoded via ACT Sign: sg = sign(1 - m1) = -sign(m1 - 1),
    so spk1 = (1 - sg)/2.  Layer-2 matmul uses lhsT ~ -0.5*w2.T, with
    the constant part reconstructed by s0/corr as above.
  - Output path: every 8 steps one ACT Sign op thresholds the m2
    sub-columns of 8 arena slots (a strided 3D AP) into fp8 bytes
    (-1 <=> spike) and one gpsimd SWDGE DMA ships the block.  16 slots
    ensure the batch only reads slots >=8 steps old, so it never blocks
    the per-step merged op.
  - Timing methodology (test.py): the whole body can be wrapped in an
    on-device hardware loop (reps=R) so one NEFF execution runs the
    kernel R times back-to-back; the wall-clock slope over R is pure
    device time, immune to the ~100ms axon dispatch/RTT noise.
"""

import numpy as np

import concourse.bacc as bacc
import concourse.mybir as mybir
import concourse.tile as tile
import concourse.dve_ops as dve_ops
from concourse.dve_ops import DveOp
from concourse.dve_spec import Spec, Src0, Src1, C0, C1, C2, lower as dve_lower
from concourse.dve_uop import DveOpSpec
from concourse.bass_utils import run_bass_kernel_spmd

F32 = mybir.dt.float32
F16 = mybir.dt.float16
F8 = mybir.dt.float8e4

B, T, F, H, C = 8192, 200, 40, 128, 5
NCORES = 8
BL = B // NCORES          # 1024 batch rows per core
BH = BL // 2              # 512 per mm1 column half
BETA, THR = 0.9, 1.0
NG = 4                    # col-tile groups for layer 2
BG = BL // NG             # 256 batch rows per col group
XB = 4                    # timesteps per x DMA batch
KX = 122                  # mm1 K rows: [xh;xl;xh] (120) + 2 ones rows
LAG = 2                   # layer-2 update lag (steps)
SLOT = BL + BG            # 1280: [m1 | m2] concat span width
NSLOT = 16                # m-state arena slots
OB = 8                    # timesteps per output block


# --------------------------------------------------------------------------
# Custom DVE op: fused SNN membrane update
# --------------------------------------------------------------------------

def _snn_ref(in0, in1, s0, s1, imm2):
    out = (
        in0.astype(np.float32) * imm2
        - (in0 > s1).astype(np.float32)
        + in1.astype(np.float32)
        + s0
    )
    return out.astype(np.float32)


def _register_snn_op() -> DveOp:
    """out = in0*imm2 - (in0 > s1) + in1 + s0"""
    name = "SNN_MEMBRANE_STEP"
    for op in dve_ops.OPS:
        if op.name == name:
            return op
    body = Src0 * C2 - (Src0 > C1) + Src1 + C0
    spec = Spec(body=body, reference=_snn_ref)
    shas = {}
    for ver in ("v3", "v4"):
        uops = dve_lower(spec, ver=ver)
        shas[ver] = DveOpSpec(name=name, opcode=0, uops=uops, rd1_en=True).sha(ver)
    op = DveOp(name, spec, subdim=False, uops_sha=shas)
    dve_ops.OPS.append(op)
    dve_ops._SUB_OPCODE_FOR_NAME[op.name] = (
        dve_ops._CUSTOM_DVE_ROW_BASE + len(dve_ops.OPS) - 1
    )
    dve_ops.CUSTOM_DVE_SPECS[op.name] = spec
    return op


SNN_OP = _register_snn_op()


# --------------------------------------------------------------------------
# Bass module
# --------------------------------------------------------------------------

def build_module(t_steps: int = T, probe: str = "", reps: int = 0):
    """reps=0: plain kernel.  reps=R>0: wrap the whole body in a hardware
    loop executing it R times back-to-back on device (used for
    dispatch-free timing; membrane state carries over between passes,
    which is timing-neutral since instruction cost is data-independent)."""
    assert t_steps % XB == 0 and t_steps % OB == 0
    tb = t_steps // XB
    nblk = t_steps // OB + 1         # +1 final partial batch (lag tail)
    nc = bacc.Bacc("TRN2", target_bir_lowering=False, debug=False)

    # x packed for the K-stacked mm1: rows 0-39 = xh, rows 40-79 = xl,
    # rows 80-119 = xh again (pairs with [wh; wh; wl] on the weight side),
    # rows 120-121 = 1.0 (bias/corr-cancel ones rows).  XB steps side by
    # side in the free dim.
    XW = XB * BL
    xq = nc.dram_tensor("xq", [tb, KX, XW], F16, kind="ExternalInput").ap()
    # w1 fp16 triple-K stack [wh; wh; wl] + 2 corr-cancel rows
    w1trip = nc.dram_tensor("w1trip", [KX, H], F16, kind="ExternalInput").ap()
    # w2 fp16 pair (padded to 32 cols)
    w2qh = nc.dram_tensor("w2qh", [H, 32], F16, kind="ExternalInput").ap()
    w2ql = nc.dram_tensor("w2ql", [H, 32], F16, kind="ExternalInput").ap()
    # per-partition s0 bias for the merged DVE op (m2 corr; cancelled on
    # m1 partitions by the ones rows)
    s0vec = nc.dram_tensor("s0vec", [128, 1], F32, kind="ExternalInput").ap()
    # -s0 broadcast to [128, BG]: pre-step PSUM m2-region init
    negcorr = nc.dram_tensor("negcorr", [128, BG], F32, kind="ExternalInput").ap()
    # out[blk, 32g+c, s*BG + j] = spk2 (fp8 -1 <=> spike) for class c,
    # m2-step t = OB*blk + s - LAG, batch b = g*BG + j
    out = nc.dram_tensor(
        "out", [nblk, 128, OB * BG], F8, kind="ExternalOutput"
    ).ap()

    with tile.TileContext(nc) as tc:
        with (
            tc.tile_pool(name="const", bufs=1) as cpool,
            tc.tile_pool(name="arena", bufs=1) as apool,
            tc.tile_pool(name="xin", bufs=6) as xpool,
            tc.tile_pool(name="sgn", bufs=6) as gpool,
            tc.tile_pool(name="stage", bufs=2) as stpool,
            tc.tile_pool(name="pspan", bufs=1, space="PSUM") as ppool,
        ):
            w1t_s = cpool.tile([KX, H], F16)
            w2qh_s = cpool.tile([H, 32], F16)
            w2ql_s = cpool.tile([H, 32], F16)
            s0_s = cpool.tile([128, 1], F32)
            negc_s = cpool.tile([128, BG], F32)
            nc.sync.dma_start(w1t_s[:], w1trip[:])
            nc.sync.dma_start(w2qh_s[:], w2qh[:])
            nc.sync.dma_start(w2ql_s[:], w2ql[:])
            nc.sync.dma_start(s0_s[:], s0vec[:])
            nc.sync.dma_start(negc_s[:], negcorr[:])

            # m-state arena: slot s (mod 16) = [m1(s) | m2(s-2)]
            arena = apool.tile([128, NSLOT * SLOT], F32, tag="arena")
            nc.gpsimd.memset(arena[:], 0.0)
            arena3d = arena.rearrange("p (s n) -> p s n", s=NSLOT)

            # PSUM concat spans [p1 | p2], ping-pong; step t uses ps[t%2]
            ps = [
                ppool.tile([128, SLOT], F32, tag=f"ps{i}", name=f"ps{i}")
                for i in range(2)
            ]
            # pre-load -s0 into both m2-regions so pre-step m2 outputs
            # (m2(-2), m2(-1)) come out exactly zero despite the s0 bias
            nc.vector.tensor_copy(ps[0][:, BL:SLOT], negc_s[:])
            nc.vector.tensor_copy(ps[1][:, BL:SLOT], negc_s[:])

            def slot(s):
                o = (s % NSLOT) * SLOT
                return arena[:, o : o + SLOT]

            def emit_body():
                for t in range(t_steps + LAG):
                    pst = ps[t % 2]

                    if t < t_steps:
                        k, s = divmod(t, XB)
                        if s == 0:
                            xt = xpool.tile([KX, XW], F16, tag="x")
                            nc.sync.dma_start(xt[:], xq[k])

                        # mm1: cur1 (+ -(corr)-deposit via ones rows) into
                        # the p1 region, two N=512 bank-aligned halves
                        for half in (0, BH):
                            nc.tensor.matmul(
                                pst[:, half : half + BH],
                                w1t_s[:],
                                xt[:, s * BL + half : s * BL + half + BH],
                                start=True, stop=True,
                            )

                    # merged membrane update:
                    #   [m1(t) | m2(t-2)] = beta*[m1(t-1) | m2(t-3)]
                    #       - ([..] > 1) + [p1(t) | p2(t-2)] + s0
                    nc.vector._custom_dve(
                        SNN_OP, out=slot(t), in0=slot(t - 1),
                        in1=pst[:, 0:SLOT],
                        s0=s0_s[:, 0:1], s1=THR, imm2=BETA,
                    )

                    if t < t_steps:
                        # sg = sign(1 - m1) (= -sign(m1-1); spk1 = (1-sg)/2)
                        sg = gpool.tile([H, BL], F16, tag="sg")
                        nc.scalar.activation(
                            sg[:], slot(t)[:, 0:BL],
                            mybir.ActivationFunctionType.Sign,
                            bias=1.0, scale=-1.0,
                        )

                        # mm2(t): cur2 deposit for merged(t+2), into the
                        # SAME psum buf's m2 region (read by t+2 = t mod 2).
                        # 4 col-groups in disjoint PE col-tiles (concurrent).
                        for g in range(NG):
                            gs = sg[:, BG * g : BG * (g + 1)]
                            o = BL + 0  # m2 region base
                            nc.tensor.matmul(
                                pst[32 * g : 32 * (g + 1), o : o + BG],
                                w2qh_s[:], gs,
                                start=True, stop=False, tile_position=(0, 32 * g),
                            )
                            nc.tensor.matmul(
                                pst[32 * g : 32 * (g + 1), o : o + BG],
                                w2ql_s[:], gs,
                                start=False, stop=True, tile_position=(0, 32 * g),
                            )

                    # batched spike output: after slot(t)=8k+7, the 8 slots
                    # (8k..8k+7) hold m2(8k-2 .. 8k+5); they are in the
                    # half-arena not written for the next 8 steps.
                    if t % OB == OB - 1 or t == t_steps + LAG - 1:
                        blk = t // OB
                        base = (blk % 2) * OB if NSLOT == 2 * OB else 0
                        stage = stpool.tile([128, OB * BG], F8, tag="st")
                        st3 = stage.rearrange("p (s n) -> p s n", s=OB)
                        nc.scalar.activation(
                            st3,
                            arena3d[:, base : base + OB, BL:SLOT],
                            mybir.ActivationFunctionType.Sign,
                            bias=THR, scale=-1.0,
                        )
                        nc.gpsimd.dma_start(out[blk], stage[:])

            if reps:
                with tc.For_i(0, reps, name="rep"):
                    emit_body()
            else:
                emit_body()

    nc.compile()
    return nc


_MODULE_CACHE: dict = {}


def _get_module(t_steps: int = T):
    if t_steps not in _MODULE_CACHE:
        _MODULE_CACHE[t_steps] = build_module(t_steps)
    return _MODULE_CACHE[t_steps]


# --------------------------------------------------------------------------
# Host-side sharding / gather
# --------------------------------------------------------------------------

def _fp16_pair(a):
    hi = a.astype(np.float16)
    lo = (a - hi.astype(np.float32)).astype(np.float16)
    return hi, lo


def make_in_maps(x, w1, b1, w2, b2, t_steps: int = T):
    x = np.asarray(x, dtype=np.float32)
    w1 = np.asarray(w1, dtype=np.float32)
    b1 = np.asarray(b1, dtype=np.float32)
    w2 = np.asarray(w2, dtype=np.float32)
    b2 = np.asarray(b2, dtype=np.float32)
    tb = t_steps // XB

    w1h, w1l = _fp16_pair(w1.T)                           # [F, H] each
    w1trip = np.zeros((KX, H), np.float16)
    w1trip[0:F] = w1h
    w1trip[F : 2 * F] = w1h
    w1trip[2 * F : 3 * F] = w1l

    w2nh, w2nl = _fp16_pair((-0.5 * w2).T)                # [H, C]
    w2qh = np.zeros((H, 32), np.float16)
    w2ql = np.zeros((H, 32), np.float16)
    w2qh[:, :C] = w2nh
    w2ql[:, :C] = w2nl
    # effective -0.5*w2.T the PE uses; corr reconstructs w2 @ spk
    w_eff = w2nh.astype(np.float32) + w2nl.astype(np.float32)
    corr = -w_eff.sum(axis=0) + b2                        # [C]

    # per-partition s0 (m2 corr on partitions 32g+c, c<C; else 0),
    # represented exactly as f32(hi)+f32(lo) of an fp16 pair so the
    # mm1 ones-rows can cancel it exactly on the m1 partitions
    corr_p = np.zeros(128, np.float64)
    for g in range(NG):
        corr_p[32 * g : 32 * g + C] = corr
    target = corr_p - b1.astype(np.float64)   # ones rows deposit -(target)
    thi = target.astype(np.float32).astype(np.float16)
    tlo = (target - thi.astype(np.float64)).astype(np.float32).astype(np.float16)
    s0 = (
        thi.astype(np.float32) + tlo.astype(np.float32)
        + b1.astype(np.float32)
    ).astype(np.float32)
    # b1 is carried inside the deposit: deposit = -(thi+tlo) = b1 - corr_s0
    # so s0 must equal corr_s0 = f32(thi)+f32(tlo) + b1
    w1trip[120] = -thi
    w1trip[121] = -tlo

    s0vec = np.ascontiguousarray(s0[:, None])
    negcorr = np.ascontiguousarray(
        np.broadcast_to(-s0[:, None], (128, BG)).astype(np.float32)
    )

    in_maps = []
    for c in range(NCORES):
        xc = x[c * BL : (c + 1) * BL, :t_steps, :]        # [BL, t, F]
        xt_ = xc.transpose(1, 2, 0)                       # [t, F, BL]
        xh16, xl16 = _fp16_pair(xt_)
        ones = np.ones((t_steps, 2, BL), np.float16)
        trip = np.concatenate([xh16, xl16, xh16, ones], axis=1)  # [t,122,BL]
        xqc = (
            trip.reshape(tb, XB, KX, BL)
            .transpose(0, 2, 1, 3)
            .reshape(tb, KX, XB * BL)
        )
        in_maps.append(
            {
                "xq": np.ascontiguousarray(xqc),
                "w1trip": w1trip,
                "w2qh": w2qh,
                "w2ql": w2ql,
                "s0vec": s0vec,
                "negcorr": negcorr,
            }
        )
    return in_maps


def postprocess(results, t_steps: int = T):
    """results: list of per-core dicts with 'out' [nblk, 128, OB*BG] fp8
    sign bytes (0xB8 = -1.0 <=> spike).  Block k slot s holds m2-step
    t = OB*k + s - LAG (validity-masked)."""
    nblk = t_steps // OB + 1
    outs = []
    for c in range(NCORES):
        r = np.asarray(results[c]["out"])
        by = r.view(np.uint8) if r.dtype != np.uint8 else r
        spk = (by == 0xB8).astype(np.float32)             # fp8 -1.0 bytes
        spk = spk.reshape(nblk, NG, 32, OB, BG)[:, :, :C]  # [k, g, c, s, j]
        # flatten (k, s) -> candidate step t = OB*k + s - LAG
        spk = spk.transpose(0, 3, 1, 4, 2).reshape(nblk * OB, BL, C)
        full = np.zeros((t_steps, BL, C), np.float32)
        for k in range(nblk):
            for s in range(OB):
                t = OB * k + s - LAG
                if 0 <= t < t_steps:
                    full[t] = spk[k * OB + s]
        outs.append(full)
    return np.concatenate(outs, axis=1)                   # [t, B, C]


def kernel(x, w1, b1, w2, b2):
    nc = _get_module(T)
    in_maps = make_in_maps(x, w1, b1, w2, b2, T)
    res = run_bass_kernel_spmd(nc, in_maps, core_ids=list(range(NCORES)))
    return postprocess(res.results, T)


# revision 13
# speedup vs baseline: 1.0984x; 1.0984x over previous
"""Trainium2 Bass kernel for nn_AudioSNN: 2-layer spiking NN (snntorch Leaky).

Reference semantics per timestep t (over T=200 steps):
    cur1 = x_t @ w1.T + b1                      # [B, 128]
    m1   = 0.9*m1 + cur1 - (m1_prev > 1)        # reset-by-subtract
    spk1 = (m1 > 1)
    cur2 = spk1 @ w2.T + b2                     # [B, 5]
    m2   = 0.9*m2 + cur2 - (m2_prev > 1)
    out[t] = spk2 = (m2 > 1)

Strategy (pure data-parallel over batch, 8 cores x 1024 batch rows):
  - Transposed layout: states kept as [feature, batch] so H=128 sits on
    SBUF partitions and batch on the free dim.
  - MERGED membrane update: ONE custom DVE op per step processes the
    concatenated [m1(1024) | m2(256)] span.  The DVE is the bottleneck
    engine (fp32 PSUM-source ops run at 1x = ~1 elem/cycle/lane), so
    fusing the two updates saves one op's fixed overhead (~130ns/step).
    Layout requirements this imposes:
      * p1 (mm1 out, [128,1024]) and p2 (mm2 out, [128,256]) live in ONE
        contiguous PSUM span [128,1280] (3 banks; 2 ping-pong bufs).
      * m-state lives in a 16-slot SBUF arena; slot s = [m1(s) | m2(s-3)].
        The 3-step layer-2 lag makes the fusion non-circular AND keeps the
        recurrence cycle merged->sign1->mm2->merged(+3) under three
        pipeline periods (lag 2 was measured cycle-limited at ~2.1us/step).
        The mm2(s) deposit goes into the mod-2 psum buf read by
        merged(s+3); emitting mm2(s) during step s+2 orders it after
        merged(s+1)'s read of that same region.
      * The m2 bias correction (corr = 0.5*sum(w2)+b2, from the +/-1 spike
        encoding) becomes the DVE op's per-partition s0 bias, which also
        (wrongly) lands on the m1 columns; two extra all-ones K-rows in
        the mm1 stack deposit exactly -(corr_hi+corr_lo) there to cancel
        it (s0 is set to float32(corr_hi)+float32(corr_lo) so the
        cancellation is exact at f32 grade).
      * Pre-step m2 slots (m2(-2), m2(-1)) must come out as zero despite
        the s0 bias: the two PSUM m2-regions are pre-loaded with -s0 via
        a one-time DMA.
  - All matmuls run in fp16 with hi/lo split pairs (x = xh + xl exactly
    to ~2^-22 rel; w likewise), accumulated exactly in fp32 PSUM:
    mm1 = wh@xh + wh@xl + wl@xh (one K=122-stacked pass incl. ones rows,
    two N=512 halves); mm2 = w2h@sg + w2l@sg (2 passes, col-tiled 4x,
    the 4 groups run concurrently in disjoint 32-col PE tiles).
  - Spikes are encoded via ACT Sign: sg = sign(1 - m1) = -sign(m1 - 1),
    so spk1 = (1 - sg)/2.  Layer-2 matmul uses lhsT ~ -0.5*w2.T, with
    the constant part reconstructed by s0/corr as above.
  - Output path: every 8 steps one ACT Sign op thresholds the m2
    sub-columns of 8 arena slots (a strided 3D AP) into fp8 bytes
    (-1 <=> spike) and one gpsimd SWDGE DMA ships the block.  16 slots
    ensure the batch only reads slots >=8 steps old, so it never blocks
    the per-step merged op.
  - Timing methodology (test.py): the whole body can be wrapped in an
    on-device hardware loop (reps=R) so one NEFF execution runs the
    kernel R times back-to-back; the wall-clock slope over R is pure
    device time, immune to the ~100ms axon dispatch/RTT noise.
"""

import numpy as np

import concourse.bacc as bacc
import concourse.mybir as mybir
import concourse.tile as tile
import concourse.dve_ops as dve_ops
from concourse.dve_ops import DveOp
from concourse.dve_spec import Spec, Src0, Src1, C0, C1, C2, lower as dve_lower
from concourse.dve_uop import DveOpSpec
from concourse.bass_utils import run_bass_kernel_spmd

F32 = mybir.dt.float32
F16 = mybir.dt.float16
F8 = mybir.dt.float8e4

B, T, F, H, C = 8192, 200, 40, 128, 5
NCORES = 8
BL = B // NCORES          # 1024 batch rows per core
BH = BL // 2              # 512 per mm1 column half
BETA, THR = 0.9, 1.0
NG = 4                    # col-tile groups for layer 2
BG = BL // NG             # 256 batch rows per col group
XB = 4                    # timesteps per x DMA batch
KX = 122                  # mm1 K rows: [xh;xl;xh] (120) + 2 ones rows
LAG = 3                   # layer-2 update lag (steps)
SLOT = BL + BG            # 1280: [m1 | m2] concat span width
NSLOT = 16                # m-state arena slots
OB = 8                    # timesteps per output block (one batched sign op)
SHIP = 3                  # output blocks per DMA (SWDGE issue is ~19us/DMA
                          # of gpsimd sequencer time regardless of size, so
                          # batch 3 blocks into each transfer)


# --------------------------------------------------------------------------
# Custom DVE op: fused SNN membrane update
# --------------------------------------------------------------------------

def _snn_ref(in0, in1, s0, s1, imm2):
    out = (
        in0.astype(np.float32) * imm2
        - (in0 > s1).astype(np.float32)
        + in1.astype(np.float32)
        + s0
    )
    return out.astype(np.float32)


def _register_snn_op() -> DveOp:
    """out = in0*imm2 - (in0 > s1) + in1 + s0"""
    name = "SNN_MEMBRANE_STEP"
    for op in dve_ops.OPS:
        if op.name == name:
            return op
    body = Src0 * C2 - (Src0 > C1) + Src1 + C0
    spec = Spec(body=body, reference=_snn_ref)
    shas = {}
    for ver in ("v3", "v4"):
        uops = dve_lower(spec, ver=ver)
        shas[ver] = DveOpSpec(name=name, opcode=0, uops=uops, rd1_en=True).sha(ver)
    op = DveOp(name, spec, subdim=False, uops_sha=shas)
    dve_ops.OPS.append(op)
    dve_ops._SUB_OPCODE_FOR_NAME[op.name] = (
        dve_ops._CUSTOM_DVE_ROW_BASE + len(dve_ops.OPS) - 1
    )
    dve_ops.CUSTOM_DVE_SPECS[op.name] = spec
    return op


SNN_OP = _register_snn_op()


# --------------------------------------------------------------------------
# Bass module
# --------------------------------------------------------------------------

def build_module(t_steps: int = T, probe: str = "", reps: int = 0):
    """reps=0: plain kernel.  reps=R>0: wrap the whole body in a hardware
    loop executing it R times back-to-back on device (used for
    dispatch-free timing; membrane state carries over between passes,
    which is timing-neutral since instruction cost is data-independent)."""
    assert t_steps % XB == 0 and t_steps % OB == 0
    tb = t_steps // XB
    nblk = t_steps // OB + 1         # +1 final partial batch (lag tail)
    nc = bacc.Bacc("TRN2", target_bir_lowering=False, debug=False)

    # x packed for the K-stacked mm1: rows 0-39 = xh, rows 40-79 = xl,
    # rows 80-119 = xh again (pairs with [wh; wh; wl] on the weight side),
    # rows 120-121 = 1.0 (bias/corr-cancel ones rows).  XB steps side by
    # side in the free dim.
    XW = XB * BL
    xq = nc.dram_tensor("xq", [tb, KX, XW], F16, kind="ExternalInput").ap()
    # w1 fp16 triple-K stack [wh; wh; wl] + 2 corr-cancel rows
    w1trip = nc.dram_tensor("w1trip", [KX, H], F16, kind="ExternalInput").ap()
    # w2 fp16 pair (padded to 32 cols)
    w2qh = nc.dram_tensor("w2qh", [H, 32], F16, kind="ExternalInput").ap()
    w2ql = nc.dram_tensor("w2ql", [H, 32], F16, kind="ExternalInput").ap()
    # per-partition s0 bias for the merged DVE op (m2 corr; cancelled on
    # m1 partitions by the ones rows)
    s0vec = nc.dram_tensor("s0vec", [128, 1], F32, kind="ExternalInput").ap()
    # -s0 broadcast to [128, BG]: pre-step PSUM m2-region init
    negcorr = nc.dram_tensor("negcorr", [128, BG], F32, kind="ExternalInput").ap()
    # out[5g+c, (blk*OB + s)*BG + j] = spk2 (fp8 -1 <=> spike) for class
    # c, m2-step t = OB*blk + s - LAG, batch b = g*BG + j.  Only the 20
    # class-bearing partitions ship (4 per-group DMAs per SHIP window);
    # partition-major so each partition row is one contiguous descriptor.
    out = nc.dram_tensor(
        "out", [NG * C, nblk * OB * BG], F8, kind="ExternalOutput"
    ).ap()

    with tile.TileContext(nc) as tc:
        with (
            tc.tile_pool(name="const", bufs=1) as cpool,
            tc.tile_pool(name="arena", bufs=1) as apool,
            tc.tile_pool(name="xin", bufs=6) as xpool,
            tc.tile_pool(name="sgn", bufs=6) as gpool,
            tc.tile_pool(name="stage", bufs=2) as stpool,
            tc.tile_pool(name="pspan", bufs=1, space="PSUM") as ppool,
        ):
            w1t_s = cpool.tile([KX, H], F16)
            w2qh_s = cpool.tile([H, 32], F16)
            w2ql_s = cpool.tile([H, 32], F16)
            s0_s = cpool.tile([128, 1], F32)
            negc_s = cpool.tile([128, BG], F32)
            nc.sync.dma_start(w1t_s[:], w1trip[:])
            nc.sync.dma_start(w2qh_s[:], w2qh[:])
            nc.sync.dma_start(w2ql_s[:], w2ql[:])
            nc.sync.dma_start(s0_s[:], s0vec[:])
            nc.sync.dma_start(negc_s[:], negcorr[:])

            # m-state arena: slot s (mod 16) = [m1(s) | m2(s-3)].  Only
            # slot NSLOT-1 (= slot(-1), read by merged(0)) needs zeroing;
            # every other slot is written before its first read.
            arena = apool.tile([128, NSLOT * SLOT], F32, tag="arena")
            nc.gpsimd.memset(arena[:, (NSLOT - 1) * SLOT :], 0.0)
            arena3d = arena.rearrange("p (s n) -> p s n", s=NSLOT)

            # PSUM concat spans [p1 | p2], ping-pong; step t uses ps[t%2]
            ps = [
                ppool.tile([128, SLOT], F32, tag=f"ps{i}", name=f"ps{i}")
                for i in range(2)
            ]
            # pre-load -s0 into both m2-regions so pre-step m2 outputs
            # (m2(-2), m2(-1)) come out exactly zero despite the s0 bias
            nc.vector.tensor_copy(ps[0][:, BL:SLOT], negc_s[:])
            nc.vector.tensor_copy(ps[1][:, BL:SLOT], negc_s[:])

            def slot(s):
                o = (s % NSLOT) * SLOT
                return arena[:, o : o + SLOT]

            def emit_body():
                sgs = {}
                state = {"stage": None}
                BW = OB * BG                       # 2048: fp8 cols per block
                for t in range(t_steps + LAG):
                    pst = ps[t % 2]

                    if t < t_steps:
                        k, s = divmod(t, XB)
                        if s == 0:
                            xt = xpool.tile([KX, XW], F16, tag="x")
                            nc.sync.dma_start(xt[:], xq[k])

                        # mm1: cur1 (+ -(corr)-deposit via ones rows) into
                        # the p1 region, two N=512 bank-aligned halves
                        for half in (0, BH):
                            nc.tensor.matmul(
                                pst[:, half : half + BH],
                                w1t_s[:],
                                xt[:, s * BL + half : s * BL + half + BH],
                                start=True, stop=True,
                            )

                    # merged membrane update:
                    #   [m1(t) | m2(t-3)] = beta*[m1(t-1) | m2(t-4)]
                    #       - ([..] > 1) + [p1(t) | p2(t-3)] + s0
                    nc.vector._custom_dve(
                        SNN_OP, out=slot(t), in0=slot(t - 1),
                        in1=pst[:, 0:SLOT],
                        s0=s0_s[:, 0:1], s1=THR, imm2=BETA,
                    )

                    if t < t_steps:
                        # sg = sign(1 - m1) (= -sign(m1-1); spk1 = (1-sg)/2)
                        sg = gpool.tile([H, BL], F16, tag="sg")
                        sgs[t] = sg
                        nc.scalar.activation(
                            sg[:], slot(t)[:, 0:BL],
                            mybir.ActivationFunctionType.Sign,
                            bias=1.0, scale=-1.0,
                        )

                    # mm2(u), u = t-2: cur2 deposit for merged(u+3), into
                    # psum buf (u+3)%2 = (t+1)%2.  Emitted two steps after
                    # sign1(u) so the WAR against merged(u+1)'s read of
                    # that m2 region resolves in program order, keeping
                    # the recurrence cycle spread over 3 periods.
                    # 4 col-groups in disjoint 32-wide PE col-tiles.
                    u = t - 2
                    if 0 <= u < t_steps:
                        psu = ps[(u + 3) % 2]
                        gsg = sgs.pop(u)
                        for g in range(NG):
                            gs = gsg[:, BG * g : BG * (g + 1)]
                            nc.tensor.matmul(
                                psu[32 * g : 32 * (g + 1), BL : BL + BG],
                                w2qh_s[:], gs,
                                start=True, stop=False, tile_position=(0, 32 * g),
                            )
                            nc.tensor.matmul(
                                psu[32 * g : 32 * (g + 1), BL : BL + BG],
                                w2ql_s[:], gs,
                                start=False, stop=True, tile_position=(0, 32 * g),
                            )

                    # batched spike output: after slot(t)=8k+7, the 8 slots
                    # (8k..8k+7) hold m2(8k-3 .. 8k+4); they are in the
                    # half-arena not written for the next 8 steps.  SHIP
                    # blocks share one staging tile / one SWDGE DMA.
                    if t % OB == OB - 1 or t == t_steps + LAG - 1:
                        blk = t // OB
                        base = (blk % 2) * OB if NSLOT == 2 * OB else 0
                        bs = blk % SHIP
                        if bs == 0:
                            state["stage"] = stpool.tile(
                                [128, SHIP * BW], F8, tag="st", name="stg"
                            )
                        stage = state["stage"]
                        st3 = stage[:, bs * BW : (bs + 1) * BW].rearrange(
                            "p (s n) -> p s n", s=OB
                        )
                        nc.scalar.activation(
                            st3,
                            arena3d[:, base : base + OB, BL:SLOT],
                            mybir.ActivationFunctionType.Sign,
                            bias=THR, scale=-1.0,
                        )
                        if bs == SHIP - 1 or blk == nblk - 1:
                            b0 = blk - bs
                            for g in range(NG):
                                nc.gpsimd.dma_start(
                                    out[C * g : C * (g + 1),
                                        b0 * BW : (blk + 1) * BW],
                                    stage[32 * g : 32 * g + C,
                                          0 : (bs + 1) * BW],
                                )

            if reps:
                with tc.For_i(0, reps, name="rep"):
                    emit_body()
            else:
                emit_body()

    nc.compile()
    return nc


_MODULE_CACHE: dict = {}


def _get_module(t_steps: int = T):
    if t_steps not in _MODULE_CACHE:
        _MODULE_CACHE[t_steps] = build_module(t_steps)
    return _MODULE_CACHE[t_steps]


# --------------------------------------------------------------------------
# Host-side sharding / gather
# --------------------------------------------------------------------------

def _fp16_pair(a):
    hi = a.astype(np.float16)
    lo = (a - hi.astype(np.float32)).astype(np.float16)
    return hi, lo


def make_in_maps(x, w1, b1, w2, b2, t_steps: int = T):
    x = np.asarray(x, dtype=np.float32)
    w1 = np.asarray(w1, dtype=np.float32)
    b1 = np.asarray(b1, dtype=np.float32)
    w2 = np.asarray(w2, dtype=np.float32)
    b2 = np.asarray(b2, dtype=np.float32)
    tb = t_steps // XB

    w1h, w1l = _fp16_pair(w1.T)                           # [F, H] each
    w1trip = np.zeros((KX, H), np.float16)
    w1trip[0:F] = w1h
    w1trip[F : 2 * F] = w1h
    w1trip[2 * F : 3 * F] = w1l

    w2nh, w2nl = _fp16_pair((-0.5 * w2).T)                # [H, C]
    w2qh = np.zeros((H, 32), np.float16)
    w2ql = np.zeros((H, 32), np.float16)
    w2qh[:, :C] = w2nh
    w2ql[:, :C] = w2nl
    # effective -0.5*w2.T the PE uses; corr reconstructs w2 @ spk
    w_eff = w2nh.astype(np.float32) + w2nl.astype(np.float32)
    corr = -w_eff.sum(axis=0) + b2                        # [C]

    # per-partition s0 (m2 corr on partitions 32g+c, c<C; else 0),
    # represented exactly as f32(hi)+f32(lo) of an fp16 pair so the
    # mm1 ones-rows can cancel it exactly on the m1 partitions
    corr_p = np.zeros(128, np.float64)
    for g in range(NG):
        corr_p[32 * g : 32 * g + C] = corr
    target = corr_p - b1.astype(np.float64)   # ones rows deposit -(target)
    thi = target.astype(np.float32).astype(np.float16)
    tlo = (target - thi.astype(np.float64)).astype(np.float32).astype(np.float16)
    s0 = (
        thi.astype(np.float32) + tlo.astype(np.float32)
        + b1.astype(np.float32)
    ).astype(np.float32)
    # b1 is carried inside the deposit: deposit = -(thi+tlo) = b1 - corr_s0
    # so s0 must equal corr_s0 = f32(thi)+f32(tlo) + b1
    w1trip[120] = -thi
    w1trip[121] = -tlo

    s0vec = np.ascontiguousarray(s0[:, None])
    negcorr = np.ascontiguousarray(
        np.broadcast_to(-s0[:, None], (128, BG)).astype(np.float32)
    )

    in_maps = []
    for c in range(NCORES):
        xc = x[c * BL : (c + 1) * BL, :t_steps, :]        # [BL, t, F]
        xt_ = xc.transpose(1, 2, 0)                       # [t, F, BL]
        xh16, xl16 = _fp16_pair(xt_)
        ones = np.ones((t_steps, 2, BL), np.float16)
        trip = np.concatenate([xh16, xl16, xh16, ones], axis=1)  # [t,122,BL]
        xqc = (
            trip.reshape(tb, XB, KX, BL)
            .transpose(0, 2, 1, 3)
            .reshape(tb, KX, XB * BL)
        )
        in_maps.append(
            {
                "xq": np.ascontiguousarray(xqc),
                "w1trip": w1trip,
                "w2qh": w2qh,
                "w2ql": w2ql,
                "s0vec": s0vec,
                "negcorr": negcorr,
            }
        )
    return in_maps


def postprocess(results, t_steps: int = T):
    """results: list of per-core dicts with 'out' [128, nblk*OB*BG] fp8
    sign bytes (0xB8 = -1.0 <=> spike).  Block k slot s holds m2-step
    t = OB*k + s - LAG (validity-masked)."""
    nblk = t_steps // OB + 1
    outs = []
    for c in range(NCORES):
        r = np.asarray(results[c]["out"])
        by = r.view(np.uint8) if r.dtype != np.uint8 else r
        spk = (by == 0xB8).astype(np.float32)             # fp8 -1.0 bytes
        spk = spk.reshape(NG, C, nblk, OB, BG)            # [g, c, k, s, j]
        # -> [k, s, b, c] with b = g*BG + j
        spk = spk.transpose(2, 3, 0, 4, 1).reshape(nblk * OB, BL, C)
        full = np.zeros((t_steps, BL, C), np.float32)
        for k in range(nblk):
            for s in range(OB):
                t = OB * k + s - LAG
                if 0 <= t < t_steps:
                    full[t] = spk[k * OB + s]
        outs.append(full)
    return np.concatenate(outs, axis=1)                   # [t, B, C]


def kernel(x, w1, b1, w2, b2):
    nc = _get_module(T)
    in_maps = make_in_maps(x, w1, b1, w2, b2, T)
    res = run_bass_kernel_spmd(nc, in_maps, core_ids=list(range(NCORES)))
    return postprocess(res.results, T)


# revision 18
# speedup vs baseline: 2.9805x; 2.7134x over previous
"""Trainium2 Bass kernel for nn_AudioSNN: 2-layer spiking NN (snntorch Leaky).

Reference semantics per timestep t (over T=200 steps):
    cur1 = x_t @ w1.T + b1                      # [B, 128]
    m1   = 0.9*m1 + cur1 - (m1_prev > 1)        # reset-by-subtract
    spk1 = (m1 > 1)
    cur2 = spk1 @ w2.T + b2                     # [B, 5]
    m2   = 0.9*m2 + cur2 - (m2_prev > 1)
    out[t] = spk2 = (m2 > 1)

Strategy (pure data-parallel over batch, 8 cores x 1024 batch rows):
  - Transposed layout: states kept as [feature, batch] so H=128 sits on
    SBUF partitions and batch on the free dim.
  - One fused custom DVE op does a whole membrane update in a single
    instruction:  m_new = m*beta - (m > thr) + cur + bias.
  - Spikes are encoded via ACT Sign: sg = sign(1 - m1) = -sign(m1 - 1),
    so spk1 = (1 - sg)/2.  Layer-2 matmul uses lhsT ~ -0.5*w2.T and a
    per-partition bias to reconstruct w2 @ spk1.
  - All matmuls run in fp16 with hi/lo split pairs (x = xh + xl exactly
    to ~2^-22 rel; w likewise), accumulated exactly in fp32 PSUM:
    mm1 = wh@xh + wh@xl + wl@xh (one K=120-stacked pass, two N=512
    halves); mm2 = w2h@sg + w2l@sg (2 passes, col-tiled 4x).
  - Output path (cheap): layer-2 membranes accumulate in a wide
    [128, OB*256] f32 staging tile; once per OB=20-step block one ACT
    Sign op thresholds the whole block into fp8 bytes (-1 <=> spike)
    and one gpsimd SWDGE DMA ships it to DRAM.  This keeps the SP
    queue free for the x-stream, moves 1 byte per (class, step,
    batch) instead of 4, and keeps compute off GPSIMD (whose ~2us
    per-op launch overhead makes per-step Pool ops prohibitive).
  - Timing methodology (test.py): the whole T-step body can be wrapped
    in an on-device hardware loop (reps=R) so one NEFF execution runs
    the kernel R times back-to-back; the wall-clock slope over R is
    pure device time, immune to the ~100ms axon dispatch/RTT noise.
"""

import numpy as np

import concourse.bacc as bacc
import concourse.mybir as mybir
import concourse.tile as tile
import concourse.dve_ops as dve_ops
from concourse.dve_ops import DveOp
from concourse.dve_spec import Spec, Src0, Src1, C0, C1, C2, lower as dve_lower
from concourse.dve_uop import DveOpSpec
from concourse.bass_utils import run_bass_kernel_spmd

F32 = mybir.dt.float32
F16 = mybir.dt.float16
F8 = mybir.dt.float8e4

B, T, F, H, C = 8192, 200, 40, 128, 5
NCORES = 8
BL = B // NCORES          # 1024 batch rows per core
BH = BL // 2              # 512 per mm1 column half
BETA, THR = 0.9, 1.0
NG = 4                    # col-tile groups for layer 2
BG = BL // NG             # 256 batch rows per col group
XB = 4                    # timesteps per x DMA batch
OB = 20                   # timesteps per output block (one batched spike
                          # op + one DMA per block: gpsimd launch overhead
                          # ~2us/op makes per-step Pool ops prohibitive)


# --------------------------------------------------------------------------
# Custom DVE op: fused SNN membrane update
# --------------------------------------------------------------------------

def _snn_ref(in0, in1, s0, s1, imm2):
    out = (
        in0.astype(np.float32) * imm2
        - (in0 > s1).astype(np.float32)
        + in1.astype(np.float32)
        + s0
    )
    return out.astype(np.float32)


def _register_snn_op() -> DveOp:
    """out = in0*imm2 - (in0 > s1) + in1 + s0"""
    name = "SNN_MEMBRANE_STEP"
    for op in dve_ops.OPS:
        if op.name == name:
            return op
    body = Src0 * C2 - (Src0 > C1) + Src1 + C0
    spec = Spec(body=body, reference=_snn_ref)
    shas = {}
    for ver in ("v3", "v4"):
        uops = dve_lower(spec, ver=ver)
        shas[ver] = DveOpSpec(name=name, opcode=0, uops=uops, rd1_en=True).sha(ver)
    op = DveOp(name, spec, subdim=False, uops_sha=shas)
    dve_ops.OPS.append(op)
    dve_ops._SUB_OPCODE_FOR_NAME[op.name] = (
        dve_ops._CUSTOM_DVE_ROW_BASE + len(dve_ops.OPS) - 1
    )
    dve_ops.CUSTOM_DVE_SPECS[op.name] = spec
    return op


SNN_OP = _register_snn_op()


# --------------------------------------------------------------------------
# Bass module
# --------------------------------------------------------------------------

def build_module(t_steps: int = T, probe: str = "", reps: int = 0):
    """reps=0: plain kernel.  reps=R>0: wrap the whole T-step body in a
    hardware loop executing it R times back-to-back on device (used for
    dispatch-free timing: one NEFF execution = R kernel passes; membrane
    state carries over between passes, which is timing-neutral since every
    instruction's cost is data-independent and values stay bounded)."""
    ob = OB if t_steps % OB == 0 else min(OB, t_steps)
    assert t_steps % XB == 0 and t_steps % ob == 0
    # buffer rotations (mod 4) must land back on the initial tiles at the
    # loop seam so rep 2+ reads the tile rep 1 last wrote
    assert reps == 0 or t_steps % 4 == 0
    tb = t_steps // XB
    ob_n = t_steps // ob
    nc = bacc.Bacc("TRN2", target_bir_lowering=False, debug=False)

    # x packed for the K-stacked 3-pass mm1: rows 0-39 = xh, rows 40-79
    # = xl, rows 80-119 = xh again (pairs with [wh; wh; wl] on the weight
    # side).  XB steps side by side in the free dim.
    XW = XB * BL
    xq = nc.dram_tensor("xq", [tb, 120, XW], F16, kind="ExternalInput").ap()
    # w1 fp16 triple-K stack [wh; wh; wl]
    w1trip = nc.dram_tensor("w1trip", [120, H], F16, kind="ExternalInput").ap()
    # w2 fp16 pair (padded to 32 cols)
    w2qh = nc.dram_tensor("w2qh", [H, 32], F16, kind="ExternalInput").ap()
    w2ql = nc.dram_tensor("w2ql", [H, 32], F16, kind="ExternalInput").ap()
    bias1 = nc.dram_tensor("bias1", [H, 1], F32, kind="ExternalInput").ap()
    bias2 = nc.dram_tensor("bias2", [128, 1], F32, kind="ExternalInput").ap()
    # out[blk, 32g+c, i2*BG + j] = spk2 (fp8 0/1) for class c, step
    # t = blk*ob + i2, batch b = g*BG + j
    out = nc.dram_tensor(
        "out", [ob_n, 128, ob * BG], F8, kind="ExternalOutput"
    ).ap()

    with tile.TileContext(nc) as tc:
        with (
            tc.tile_pool(name="const", bufs=1) as cpool,
            tc.tile_pool(name="state", bufs=1) as spool,
            tc.tile_pool(name="xin", bufs=8) as xpool,
            tc.tile_pool(name="sgn", bufs=6) as gpool,
            tc.tile_pool(name="stage", bufs=2) as stpool,
            tc.tile_pool(name="m2st", bufs=2) as m2pool,
            tc.tile_pool(name="ps1", bufs=3, space="PSUM") as p1pool,
            tc.tile_pool(name="ps2", bufs=2, space="PSUM") as p2pool,
        ):
            w1t_s = cpool.tile([120, H], F16)
            w2qh_s = cpool.tile([H, 32], F16)
            w2ql_s = cpool.tile([H, 32], F16)
            b1_s = cpool.tile([H, 1], F32)
            b2_s = cpool.tile([128, 1], F32)
            nc.sync.dma_start(w1t_s[:], w1trip[:])
            nc.sync.dma_start(w2qh_s[:], w2qh[:])
            nc.sync.dma_start(w2ql_s[:], w2ql[:])
            nc.sync.dma_start(b1_s[:], bias1[:])
            nc.sync.dma_start(b2_s[:], bias2[:])

            m1_pool_prev = spool.tile([H, BL], F32, tag="m1a")
            nc.gpsimd.memset(m1_pool_prev[:], 0.0)
            m1_pool_alt = spool.tile([H, BL], F32, tag="m1b")
            m1_pool_alt2 = spool.tile([H, BL], F32, tag="m1c")
            m1_pool_alt3 = spool.tile([H, BL], F32, tag="m1d")
            m1_bufs = [m1_pool_alt, m1_pool_alt2, m1_pool_alt3, m1_pool_prev]
            m1_prev = m1_pool_prev

            # layer-2 membranes accumulate into a wide per-block staging
            # tile; one batched fp8 spike op + one DMA ships each block
            m2st_prev = spool.tile([128, BG], F32, tag="m2i")
            nc.gpsimd.memset(m2st_prev[:], 0.0)

            p1_st = p2_st = x_st = sg_st = None
            if probe == "no_mm1":
                p1_st = spool.tile([H, BL], F32, tag="p1s")
                nc.gpsimd.memset(p1_st[:], 0.1)
            if probe == "no_mm2":
                p2_st = spool.tile([128, BG], F32, tag="p2s")
                nc.gpsimd.memset(p2_st[:], 0.1)
            if probe == "no_xdma":
                x_st = spool.tile([120, XW], F16, tag="xs")
                nc.sync.dma_start(x_st[:], xq[0])
            if probe == "no_act":
                sg_st = spool.tile([H, BL], F16, tag="sgs")
                nc.gpsimd.memset(sg_st[:], 1.0)

            state = {"m2_prev": m2st_prev[:], "m2st": None, "p2": None}

            def l2_step(tau):
                """Membrane-2 update for step tau (runs one step late so
                the DVE queue never stalls on the ACT->PE chain); at block
                end one batched gpsimd op thresholds the whole block to
                fp8 spikes and one SWDGE DMA ships it."""
                i2 = tau % ob
                if i2 == 0:
                    state["m2st"] = m2pool.tile(
                        [128, ob * BG], F32, tag="m2st", name="m2st"
                    )
                m2 = state["m2st"][:, i2 * BG : (i2 + 1) * BG]
                if probe == "dve_std":
                    nc.vector.scalar_tensor_tensor(
                        out=m2, in0=state["m2_prev"], scalar=BETA,
                        in1=state["p2"][:],
                        op0=mybir.AluOpType.mult, op1=mybir.AluOpType.add,
                    )
                elif probe != "no_dve":
                    nc.vector._custom_dve(
                        SNN_OP, out=m2, in0=state["m2_prev"],
                        in1=state["p2"][:],
                        s0=b2_s[:, 0:1], s1=THR, imm2=BETA,
                    )
                state["m2_prev"] = m2
                # batched spike: s = sign(1 - m2) in fp8 (-1 <=> spike;
                # host decodes byte == 0xB8) on ACT, in two half-block ops
                # so they interleave with the per-step sign chain; gpsimd's
                # ~2us per-op launch overhead rules out Pool compute here
                hb = (ob // 2) * BG
                if i2 == ob // 2 - 1 and probe not in ("no_spk", "no_dve"):
                    state["stage"] = stpool.tile(
                        [128, ob * BG], F8, tag="st", name="stg"
                    )
                    nc.scalar.activation(
                        state["stage"][:, :hb], state["m2st"][:, :hb],
                        mybir.ActivationFunctionType.Sign,
                        bias=THR, scale=-1.0,
                    )
                if i2 == ob - 1 and probe not in ("no_spk", "no_dve"):
                    nc.scalar.activation(
                        state["stage"][:, hb:], state["m2st"][:, hb:],
                        mybir.ActivationFunctionType.Sign,
                        bias=THR, scale=-1.0,
                    )
                    if probe != "no_outdma":
                        nc.gpsimd.dma_start(out[tau // ob], state["stage"][:])

            from contextlib import nullcontext

            def emit_body():
                nonlocal m1_prev
                for t in range(t_steps):
                    k, s = divmod(t, XB)

                    if s == 0:
                        if probe == "no_xdma":
                            xt = x_st
                        else:
                            xt = xpool.tile([120, XW], F16, tag="x")
                            nc.sync.dma_start(xt[:], xq[k])

                    # mm1: cur1 = w1 @ x via one K=120 stacked pass
                    # ([wh; wh; wl] . [xh; xl; xh]), two N=512 halves
                    p1 = (
                        p1_st if probe == "no_mm1"
                        else p1pool.tile([H, BL], F32, tag="p1")
                    )
                    if probe != "no_mm1":
                        for half in (0, BH):
                            nc.tensor.matmul(
                                p1[:, half : half + BH],
                                w1t_s[:],
                                xt[:, s * BL + half : s * BL + half + BH],
                                start=True, stop=True,
                            )

                    # m1 = beta*m1 - (m1 > 1) + cur1 + b1  (ping-pong
                    # buffers so the next step's write doesn't WAR-wait
                    # on ACT's read)
                    m1 = m1_bufs[3] if probe == "no_dve" else m1_bufs[t % 4]
                    if probe == "dve_std":
                        nc.vector.scalar_tensor_tensor(
                            out=m1[:], in0=m1_prev[:], scalar=BETA, in1=p1[:],
                            op0=mybir.AluOpType.mult, op1=mybir.AluOpType.add,
                        )
                    elif probe != "no_dve":
                        nc.vector._custom_dve(
                            SNN_OP, out=m1[:], in0=m1_prev[:], in1=p1[:],
                            s0=b1_s[:, 0:1], s1=THR, imm2=BETA,
                        )
                    m1_prev = m1

                    # sg = sign(1 - m1) (= -sign(m1-1); spk1 = (1-sg)/2)
                    if probe == "no_act":
                        sg = sg_st
                    else:
                        sg = gpool.tile([H, BL], F16, tag="sg")
                        nc.scalar.activation(
                            sg[:], m1[:], mybir.ActivationFunctionType.Sign,
                            bias=1.0, scale=-1.0,
                        )

                    # cur2: p2[32g+c, j] = -0.5*(w2@sgn1)[c, 256g+j], 2-pass
                    p2 = (
                        p2_st if probe == "no_mm2"
                        else p2pool.tile([128, BG], F32, tag="p2")
                    )
                    for g in () if probe == "no_mm2" else range(NG):
                        gs = sg[:, BG * g : BG * (g + 1)]
                        nc.tensor.matmul(
                            p2[32 * g : 32 * (g + 1), :], w2qh_s[:], gs,
                            start=True, stop=False, tile_position=(0, 32 * g),
                        )
                        nc.tensor.matmul(
                            p2[32 * g : 32 * (g + 1), :], w2ql_s[:], gs,
                            start=False, stop=True, tile_position=(0, 32 * g),
                        )

                    # m2(t-1) update, one step behind
                    if t > 0:
                        l2_step(t - 1)
                    state["p2"] = p2

                l2_step(t_steps - 1)

            if reps:
                with tc.For_i(0, reps, name="rep"):
                    emit_body()
            else:
                emit_body()

    nc.compile()
    return nc


_MODULE_CACHE: dict = {}


def _get_module(t_steps: int = T):
    if t_steps not in _MODULE_CACHE:
        _MODULE_CACHE[t_steps] = build_module(t_steps)
    return _MODULE_CACHE[t_steps]


# --------------------------------------------------------------------------
# Host-side sharding / gather
# --------------------------------------------------------------------------

def _fp16_pair(a):
    hi = a.astype(np.float16)
    lo = (a - hi.astype(np.float32)).astype(np.float16)
    return hi, lo


def make_in_maps(x, w1, b1, w2, b2, t_steps: int = T):
    x = np.asarray(x, dtype=np.float32)
    w1 = np.asarray(w1, dtype=np.float32)
    b1 = np.asarray(b1, dtype=np.float32)
    w2 = np.asarray(w2, dtype=np.float32)
    b2 = np.asarray(b2, dtype=np.float32)
    tb = t_steps // XB

    w1h, w1l = _fp16_pair(w1.T)                           # [F, H] each
    w1trip = np.zeros((120, H), np.float16)
    w1trip[0:F] = w1h
    w1trip[F : 2 * F] = w1h
    w1trip[2 * F : 3 * F] = w1l

    w2nh, w2nl = _fp16_pair((-0.5 * w2).T)                # [H, C]
    w2qh = np.zeros((H, 32), np.float16)
    w2ql = np.zeros((H, 32), np.float16)
    w2qh[:, :C] = w2nh
    w2ql[:, :C] = w2nl
    # effective -0.5*w2.T the PE uses; bias reconstructs w2 @ spk
    w_eff = w2nh.astype(np.float32) + w2nl.astype(np.float32)
    corr = -w_eff.sum(axis=0) + b2

    bias1 = np.ascontiguousarray(b1[:, None])
    bias2 = np.zeros((128, 1), np.float32)
    for g in range(NG):
        bias2[32 * g : 32 * g + C, 0] = corr

    in_maps = []
    for c in range(NCORES):
        xc = x[c * BL : (c + 1) * BL, :t_steps, :]        # [BL, t, F]
        xt_ = xc.transpose(1, 2, 0)                       # [t, F, BL]
        xh16, xl16 = _fp16_pair(xt_)
        trip = np.concatenate([xh16, xl16, xh16], axis=1)  # [t, 120, BL]
        xqc = (
            trip.reshape(tb, XB, 120, BL)
            .transpose(0, 2, 1, 3)
            .reshape(tb, 120, XB * BL)
        )
        in_maps.append(
            {
                "xq": np.ascontiguousarray(xqc),
                "w1trip": w1trip,
                "w2qh": w2qh,
                "w2ql": w2ql,
                "bias1": bias1,
                "bias2": bias2,
            }
        )
    return in_maps


def postprocess(results, t_steps: int = T):
    """results: list of per-core dicts with 'out' [ob_n, 128, OB*BG] fp8
    spikes (0/1 bytes)."""
    outs = []
    for c in range(NCORES):
        r = np.asarray(results[c]["out"])
        by = r.view(np.uint8) if r.dtype != np.uint8 else r
        spk = (by == 0xB8).astype(np.float32)             # fp8 -1.0 bytes
        ob_n = t_steps // OB
        spk = spk.reshape(ob_n, NG, 32, OB, BG)[:, :, :C]  # [ob, g, c, i2, j]
        # -> [t, b, c] with t = ob*OB+i2, b = g*BG+j
        spk = spk.transpose(0, 3, 1, 4, 2).reshape(t_steps, BL, C)
        outs.append(spk)
    return np.concatenate(outs, axis=1)                   # [t, B, C]


def kernel(x, w1, b1, w2, b2):
    nc = _get_module(T)
    in_maps = make_in_maps(x, w1, b1, w2, b2, T)
    res = run_bass_kernel_spmd(nc, in_maps, core_ids=list(range(NCORES)))
    return postprocess(res.results, T)



# revision 19
# speedup vs baseline: 2.9936x; 1.0044x over previous
"""Trainium2 Bass kernel for nn_AudioSNN: 2-layer spiking NN (snntorch Leaky).

Reference semantics per timestep t (over T=200 steps):
    cur1 = x_t @ w1.T + b1                      # [B, 128]
    m1   = 0.9*m1 + cur1 - (m1_prev > 1)        # reset-by-subtract
    spk1 = (m1 > 1)
    cur2 = spk1 @ w2.T + b2                     # [B, 5]
    m2   = 0.9*m2 + cur2 - (m2_prev > 1)
    out[t] = spk2 = (m2 > 1)

Strategy (pure data-parallel over batch, 8 cores x 1024 batch rows):
  - Transposed layout: states kept as [feature, batch] so H=128 sits on
    SBUF partitions and batch on the free dim.
  - One fused custom DVE op does a whole membrane update in a single
    instruction:  m_new = m*beta - (m > thr) + cur + bias.
  - Spikes are encoded via ACT Sign: sg = sign(1 - m1) = -sign(m1 - 1),
    so spk1 = (1 - sg)/2.  Layer-2 matmul uses lhsT ~ -0.5*w2.T and a
    per-partition bias to reconstruct w2 @ spk1.
  - All matmuls run in fp16 with hi/lo split pairs (x = xh + xl exactly
    to ~2^-22 rel; w likewise), accumulated exactly in fp32 PSUM:
    mm1 = wh@xh + wh@xl + wl@xh (one K=120-stacked pass, two N=512
    halves); mm2 = w2h@sg + w2l@sg (2 passes, col-tiled 4x).
  - Output path (cheap): layer-2 membranes accumulate in a wide
    [128, OB*256] f32 staging tile; once per OB=20-step block one ACT
    Sign op thresholds the whole block into fp8 bytes (-1 <=> spike)
    and one gpsimd SWDGE DMA ships it to DRAM.  This keeps the SP
    queue free for the x-stream, moves 1 byte per (class, step,
    batch) instead of 4, and keeps compute off GPSIMD (whose ~2us
    per-op launch overhead makes per-step Pool ops prohibitive).
  - Timing methodology (test.py): the whole T-step body can be wrapped
    in an on-device hardware loop (reps=R) so one NEFF execution runs
    the kernel R times back-to-back; the wall-clock slope over R is
    pure device time, immune to the ~100ms axon dispatch/RTT noise.
"""

import numpy as np

import concourse.bacc as bacc
import concourse.mybir as mybir
import concourse.tile as tile
import concourse.dve_ops as dve_ops
from concourse.dve_ops import DveOp
from concourse.dve_spec import Spec, Src0, Src1, C0, C1, C2, lower as dve_lower
from concourse.dve_uop import DveOpSpec
from concourse.bass_utils import run_bass_kernel_spmd

F32 = mybir.dt.float32
F16 = mybir.dt.float16
F8 = mybir.dt.float8e4

B, T, F, H, C = 8192, 200, 40, 128, 5
NCORES = 8
BL = B // NCORES          # 1024 batch rows per core
BH = BL // 2              # 512 per mm1 column half
BETA, THR = 0.9, 1.0
NG = 4                    # col-tile groups for layer 2
BG = BL // NG             # 256 batch rows per col group
XB = 4                    # timesteps per x DMA batch
OB = 20                   # timesteps per output block (one batched spike
                          # op + one DMA per block: gpsimd launch overhead
                          # ~2us/op makes per-step Pool ops prohibitive)


# --------------------------------------------------------------------------
# Custom DVE op: fused SNN membrane update
# --------------------------------------------------------------------------

def _snn_ref(in0, in1, s0, s1, imm2):
    out = (
        in0.astype(np.float32) * imm2
        - (in0 > s1).astype(np.float32)
        + in1.astype(np.float32)
        + s0
    )
    return out.astype(np.float32)


def _register_snn_op() -> DveOp:
    """out = in0*imm2 - (in0 > s1) + in1 + s0"""
    name = "SNN_MEMBRANE_STEP"
    for op in dve_ops.OPS:
        if op.name == name:
            return op
    body = Src0 * C2 - (Src0 > C1) + Src1 + C0
    spec = Spec(body=body, reference=_snn_ref)
    shas = {}
    for ver in ("v3", "v4"):
        uops = dve_lower(spec, ver=ver)
        shas[ver] = DveOpSpec(name=name, opcode=0, uops=uops, rd1_en=True).sha(ver)
    op = DveOp(name, spec, subdim=False, uops_sha=shas)
    dve_ops.OPS.append(op)
    dve_ops._SUB_OPCODE_FOR_NAME[op.name] = (
        dve_ops._CUSTOM_DVE_ROW_BASE + len(dve_ops.OPS) - 1
    )
    dve_ops.CUSTOM_DVE_SPECS[op.name] = spec
    return op


SNN_OP = _register_snn_op()


# --------------------------------------------------------------------------
# Bass module
# --------------------------------------------------------------------------

def build_module(t_steps: int = T, probe: str = "", reps: int = 0):
    """reps=0: plain kernel.  reps=R>0: wrap the whole T-step body in a
    hardware loop executing it R times back-to-back on device (used for
    dispatch-free timing: one NEFF execution = R kernel passes; membrane
    state carries over between passes, which is timing-neutral since every
    instruction's cost is data-independent and values stay bounded)."""
    ob = OB if t_steps % OB == 0 else min(OB, t_steps)
    assert t_steps % XB == 0 and t_steps % ob == 0
    # buffer rotations (mod 4) must land back on the initial tiles at the
    # loop seam so rep 2+ reads the tile rep 1 last wrote
    assert reps == 0 or t_steps % 20 == 0
    tb = t_steps // XB
    ob_n = t_steps // ob
    nc = bacc.Bacc("TRN2", target_bir_lowering=False, debug=False)

    # x packed for the K-stacked 3-pass mm1: rows 0-39 = xh, rows 40-79
    # = xl, rows 80-119 = xh again (pairs with [wh; wh; wl] on the weight
    # side).  XB steps side by side in the free dim.
    XW = XB * BL
    xq = nc.dram_tensor("xq", [tb, 120, XW], F16, kind="ExternalInput").ap()
    # w1 fp16 triple-K stack [wh; wh; wl]
    w1trip = nc.dram_tensor("w1trip", [120, H], F16, kind="ExternalInput").ap()
    # w2 fp16 pair (padded to 32 cols)
    w2qh = nc.dram_tensor("w2qh", [H, 32], F16, kind="ExternalInput").ap()
    w2ql = nc.dram_tensor("w2ql", [H, 32], F16, kind="ExternalInput").ap()
    bias1 = nc.dram_tensor("bias1", [H, 1], F32, kind="ExternalInput").ap()
    bias2 = nc.dram_tensor("bias2", [128, 1], F32, kind="ExternalInput").ap()
    # out[blk, 32g+c, i2*BG + j] = spk2 (fp8 0/1) for class c, step
    # t = blk*ob + i2, batch b = g*BG + j
    out = nc.dram_tensor(
        "out", [ob_n, 128, ob * BG], F8, kind="ExternalOutput"
    ).ap()

    with tile.TileContext(nc) as tc:
        with (
            tc.tile_pool(name="const", bufs=1) as cpool,
            tc.tile_pool(name="state", bufs=1) as spool,
            tc.tile_pool(name="xin", bufs=8) as xpool,
            tc.tile_pool(name="sgn", bufs=8) as gpool,
            tc.tile_pool(name="stage", bufs=2) as stpool,
            tc.tile_pool(name="m2st", bufs=2) as m2pool,
            tc.tile_pool(name="ps1", bufs=3, space="PSUM") as p1pool,
            tc.tile_pool(name="ps2", bufs=2, space="PSUM") as p2pool,
        ):
            w1t_s = cpool.tile([120, H], F16)
            w2qh_s = cpool.tile([H, 32], F16)
            w2ql_s = cpool.tile([H, 32], F16)
            b1_s = cpool.tile([H, 1], F32)
            b2_s = cpool.tile([128, 1], F32)
            nc.sync.dma_start(w1t_s[:], w1trip[:])
            nc.sync.dma_start(w2qh_s[:], w2qh[:])
            nc.sync.dma_start(w2ql_s[:], w2ql[:])
            nc.sync.dma_start(b1_s[:], bias1[:])
            nc.sync.dma_start(b2_s[:], bias2[:])

            m1_pool_prev = spool.tile([H, BL], F32, tag="m1a")
            nc.gpsimd.memset(m1_pool_prev[:], 0.0)
            m1_pool_alt = spool.tile([H, BL], F32, tag="m1b")
            m1_pool_alt2 = spool.tile([H, BL], F32, tag="m1c")
            m1_pool_alt3 = spool.tile([H, BL], F32, tag="m1d")
            m1_pool_alt4 = spool.tile([H, BL], F32, tag="m1e")
            m1_bufs = [m1_pool_alt, m1_pool_alt2, m1_pool_alt3, m1_pool_alt4,
                       m1_pool_prev]
            m1_prev = m1_pool_prev

            # layer-2 membranes accumulate into a wide per-block staging
            # tile; one batched fp8 spike op + one DMA ships each block
            m2st_prev = spool.tile([128, BG], F32, tag="m2i")
            nc.gpsimd.memset(m2st_prev[:], 0.0)

            p1_st = p2_st = x_st = sg_st = None
            if probe == "no_mm1":
                p1_st = spool.tile([H, BL], F32, tag="p1s")
                nc.gpsimd.memset(p1_st[:], 0.1)
            if probe == "no_mm2":
                p2_st = spool.tile([128, BG], F32, tag="p2s")
                nc.gpsimd.memset(p2_st[:], 0.1)
            if probe == "no_xdma":
                x_st = spool.tile([120, XW], F16, tag="xs")
                nc.sync.dma_start(x_st[:], xq[0])
            if probe == "no_act":
                sg_st = spool.tile([H, BL], F16, tag="sgs")
                nc.gpsimd.memset(sg_st[:], 1.0)

            state = {"m2_prev": m2st_prev[:], "m2st": None, "p2": None}

            def l2_step(tau):
                """Membrane-2 update for step tau (runs one step late so
                the DVE queue never stalls on the ACT->PE chain); at block
                end one batched gpsimd op thresholds the whole block to
                fp8 spikes and one SWDGE DMA ships it."""
                i2 = tau % ob
                if i2 == 0:
                    state["m2st"] = m2pool.tile(
                        [128, ob * BG], F32, tag="m2st", name="m2st"
                    )
                m2 = state["m2st"][:, i2 * BG : (i2 + 1) * BG]
                if probe == "dve_std":
                    nc.vector.scalar_tensor_tensor(
                        out=m2, in0=state["m2_prev"], scalar=BETA,
                        in1=state["p2"][:],
                        op0=mybir.AluOpType.mult, op1=mybir.AluOpType.add,
                    )
                elif probe != "no_dve":
                    nc.vector._custom_dve(
                        SNN_OP, out=m2, in0=state["m2_prev"],
                        in1=state["p2"][:],
                        s0=b2_s[:, 0:1], s1=THR, imm2=BETA,
                    )
                state["m2_prev"] = m2
                # batched spike: s = sign(1 - m2) in fp8 (-1 <=> spike;
                # host decodes byte == 0xB8) on ACT, in two half-block ops
                # so they interleave with the per-step sign chain; gpsimd's
                # ~2us per-op launch overhead rules out Pool compute here
                hb = (ob // 2) * BG
                if i2 == ob // 2 - 1 and probe not in ("no_spk", "no_dve"):
                    state["stage"] = stpool.tile(
                        [128, ob * BG], F8, tag="st", name="stg"
                    )
                    nc.scalar.activation(
                        state["stage"][:, :hb], state["m2st"][:, :hb],
                        mybir.ActivationFunctionType.Sign,
                        bias=THR, scale=-1.0,
                    )
                if i2 == ob - 1 and probe not in ("no_spk", "no_dve"):
                    nc.scalar.activation(
                        state["stage"][:, hb:], state["m2st"][:, hb:],
                        mybir.ActivationFunctionType.Sign,
                        bias=THR, scale=-1.0,
                    )
                    if probe != "no_outdma":
                        nc.gpsimd.dma_start(out[tau // ob], state["stage"][:])

            from contextlib import nullcontext

            def emit_body():
                nonlocal m1_prev
                for t in range(t_steps):
                    k, s = divmod(t, XB)

                    if s == 0:
                        if probe == "no_xdma":
                            xt = x_st
                        else:
                            xt = xpool.tile([120, XW], F16, tag="x")
                            nc.sync.dma_start(xt[:], xq[k])

                    # mm1: cur1 = w1 @ x via one K=120 stacked pass
                    # ([wh; wh; wl] . [xh; xl; xh]), two N=512 halves
                    p1 = (
                        p1_st if probe == "no_mm1"
                        else p1pool.tile([H, BL], F32, tag="p1")
                    )
                    if probe != "no_mm1":
                        for half in (0, BH):
                            nc.tensor.matmul(
                                p1[:, half : half + BH],
                                w1t_s[:],
                                xt[:, s * BL + half : s * BL + half + BH],
                                start=True, stop=True,
                            )

                    # m1 = beta*m1 - (m1 > 1) + cur1 + b1  (ping-pong
                    # buffers so the next step's write doesn't WAR-wait
                    # on ACT's read)
                    m1 = m1_bufs[-1] if probe == "no_dve" else m1_bufs[t % 5]
                    if probe == "dve_std":
                        nc.vector.scalar_tensor_tensor(
                            out=m1[:], in0=m1_prev[:], scalar=BETA, in1=p1[:],
                            op0=mybir.AluOpType.mult, op1=mybir.AluOpType.add,
                        )
                    elif probe != "no_dve":
                        nc.vector._custom_dve(
                            SNN_OP, out=m1[:], in0=m1_prev[:], in1=p1[:],
                            s0=b1_s[:, 0:1], s1=THR, imm2=BETA,
                        )
                    m1_prev = m1

                    # sg = sign(1 - m1) (= -sign(m1-1); spk1 = (1-sg)/2)
                    if probe == "no_act":
                        sg = sg_st
                    else:
                        sg = gpool.tile([H, BL], F16, tag="sg")
                        nc.scalar.activation(
                            sg[:], m1[:], mybir.ActivationFunctionType.Sign,
                            bias=1.0, scale=-1.0,
                        )

                    # cur2: p2[32g+c, j] = -0.5*(w2@sgn1)[c, 256g+j], 2-pass
                    p2 = (
                        p2_st if probe == "no_mm2"
                        else p2pool.tile([128, BG], F32, tag="p2")
                    )
                    for g in () if probe == "no_mm2" else range(NG):
                        gs = sg[:, BG * g : BG * (g + 1)]
                        nc.tensor.matmul(
                            p2[32 * g : 32 * (g + 1), :], w2qh_s[:], gs,
                            start=True, stop=False, tile_position=(0, 32 * g),
                        )
                        nc.tensor.matmul(
                            p2[32 * g : 32 * (g + 1), :], w2ql_s[:], gs,
                            start=False, stop=True, tile_position=(0, 32 * g),
                        )

                    # m2(t-1) update, one step behind
                    if t > 0:
                        l2_step(t - 1)
                    state["p2"] = p2

                l2_step(t_steps - 1)

            if reps:
                with tc.For_i(0, reps, name="rep"):
                    emit_body()
            else:
                emit_body()

    nc.compile()
    return nc


_MODULE_CACHE: dict = {}


def _get_module(t_steps: int = T):
    if t_steps not in _MODULE_CACHE:
        _MODULE_CACHE[t_steps] = build_module(t_steps)
    return _MODULE_CACHE[t_steps]


# --------------------------------------------------------------------------
# Host-side sharding / gather
# --------------------------------------------------------------------------

def _fp16_pair(a):
    hi = a.astype(np.float16)
    lo = (a - hi.astype(np.float32)).astype(np.float16)
    return hi, lo


def make_in_maps(x, w1, b1, w2, b2, t_steps: int = T):
    x = np.asarray(x, dtype=np.float32)
    w1 = np.asarray(w1, dtype=np.float32)
    b1 = np.asarray(b1, dtype=np.float32)
    w2 = np.asarray(w2, dtype=np.float32)
    b2 = np.asarray(b2, dtype=np.float32)
    tb = t_steps // XB

    w1h, w1l = _fp16_pair(w1.T)                           # [F, H] each
    w1trip = np.zeros((120, H), np.float16)
    w1trip[0:F] = w1h
    w1trip[F : 2 * F] = w1h
    w1trip[2 * F : 3 * F] = w1l

    w2nh, w2nl = _fp16_pair((-0.5 * w2).T)                # [H, C]
    w2qh = np.zeros((H, 32), np.float16)
    w2ql = np.zeros((H, 32), np.float16)
    w2qh[:, :C] = w2nh
    w2ql[:, :C] = w2nl
    # effective -0.5*w2.T the PE uses; bias reconstructs w2 @ spk
    w_eff = w2nh.astype(np.float32) + w2nl.astype(np.float32)
    corr = -w_eff.sum(axis=0) + b2

    bias1 = np.ascontiguousarray(b1[:, None])
    bias2 = np.zeros((128, 1), np.float32)
    for g in range(NG):
        bias2[32 * g : 32 * g + C, 0] = corr

    in_maps = []
    for c in range(NCORES):
        xc = x[c * BL : (c + 1) * BL, :t_steps, :]        # [BL, t, F]
        xt_ = xc.transpose(1, 2, 0)                       # [t, F, BL]
        xh16, xl16 = _fp16_pair(xt_)
        trip = np.concatenate([xh16, xl16, xh16], axis=1)  # [t, 120, BL]
        xqc = (
            trip.reshape(tb, XB, 120, BL)
            .transpose(0, 2, 1, 3)
            .reshape(tb, 120, XB * BL)
        )
        in_maps.append(
            {
                "xq": np.ascontiguousarray(xqc),
                "w1trip": w1trip,
                "w2qh": w2qh,
                "w2ql": w2ql,
                "bias1": bias1,
                "bias2": bias2,
            }
        )
    return in_maps


def postprocess(results, t_steps: int = T):
    """results: list of per-core dicts with 'out' [ob_n, 128, OB*BG] fp8
    spikes (0/1 bytes)."""
    outs = []
    for c in range(NCORES):
        r = np.asarray(results[c]["out"])
        by = r.view(np.uint8) if r.dtype != np.uint8 else r
        spk = (by == 0xB8).astype(np.float32)             # fp8 -1.0 bytes
        ob_n = t_steps // OB
        spk = spk.reshape(ob_n, NG, 32, OB, BG)[:, :, :C]  # [ob, g, c, i2, j]
        # -> [t, b, c] with t = ob*OB+i2, b = g*BG+j
        spk = spk.transpose(0, 3, 1, 4, 2).reshape(t_steps, BL, C)
        outs.append(spk)
    return np.concatenate(outs, axis=1)                   # [t, B, C]


def kernel(x, w1, b1, w2, b2):
    nc = _get_module(T)
    in_maps = make_in_maps(x, w1, b1, w2, b2, T)
    res = run_bass_kernel_spmd(nc, in_maps, core_ids=list(range(NCORES)))
    return postprocess(res.results, T)

